# revision 2
# baseline (speedup 1.0000x reference)
"""Trainium2 Bass kernel for batched single-head attention.

Reference computation (shapes hardcoded):
    x: [B=4, E=128, S=4096], Wq/Wk/Wv: [E,E], bq/bk/bv: [E]
    xt = x.swapaxes(1,2)                      # [B,S,E]
    q = xt @ Wq.T + bq ; k,v likewise
    out = softmax(q @ k.T / sqrt(E)) @ v      # [B,S,E]

Sharding: 8 cores = 4 batches x 2 sequence-chunks of 2048 Q rows.
Each core receives x[b] in full (for K/V) plus its Q column chunk, and
computes in fully "transposed" layouts so no on-chip transposes are
needed:
    qT = (Wq.T/sqrt(E)).T @ x_chunk          (E on partitions)
    kT = Wk @ x[b], v[t,e] per 128-row tile  (PE matmuls)
    scoresT[t,s] tiles = kT_tile.T @ qT      (PE)
    p = exp(scoresT)                         (ACT, PSUM->SBUF)
    outT += v_tile.T @ p ; denom += ones.T @ p   (PE, PSUM accumulate)
    out = outT * (1/denom)                   (DVE)
Softmax max-subtraction is skipped (scores ~ N(0,1); exp is safe in
fp32) and the V bias is added on the host (softmax rows sum to 1).
"""

import os
import sys

for _p in ("/opt/trn_rl_repo", "/root/.axon_site/_ro/trn_rl_repo"):
    if os.path.isdir(_p):
        if _p not in sys.path:
            sys.path.insert(0, _p)
        break

import numpy as np

B, E, S = 4, 128, 4096
NCORES = 8
CHUNK = 2048  # q rows per core
SBLK = 512  # moving-dim block (one PSUM bank of fp32)
NSB = CHUNK // SBLK  # 4
NT = S // 128  # 32 key/value tiles
SCALE = 1.0 / np.sqrt(E)

# "fast": fp16 operands for the three big matmul groups (~1e-3 rel err)
# "safe": fp32 matmuls (4 cycles/row on PE, ~2.5x slower, ~1.5e-4 rel err)
PRECISION = os.environ.get("BASS_ATTN_PRECISION", "fast")

_CACHE = {}


def _build_nc():
    import concourse.bacc as bacc
    import concourse.mybir as mybir
    from concourse.tile import TileContext

    f32 = mybir.dt.float32
    f16 = mybir.dt.float16
    lp = f16 if PRECISION == "fast" else f32
    Act = mybir.ActivationFunctionType

    nc = bacc.Bacc(
        "TRN2",
        target_bir_lowering=False,
        debug=False,
        enable_asserts=True,
        num_devices=NCORES,
    )

    xb = nc.dram_tensor("xb", [E, S], f32, kind="ExternalInput")
    xq = nc.dram_tensor("xq", [E, CHUNK], f32, kind="ExternalInput")
    wq = nc.dram_tensor("wq", [E, E], f32, kind="ExternalInput")  # Wq.T*SCALE
    wk = nc.dram_tensor("wk", [E, E], f32, kind="ExternalInput")  # Wk.T
    wv = nc.dram_tensor("wv", [E, E], f32, kind="ExternalInput")  # Wv.T
    bq = nc.dram_tensor("bq", [E, 1], f32, kind="ExternalInput")  # bq*SCALE
    bk = nc.dram_tensor("bk", [E, 1], f32, kind="ExternalInput")
    out = nc.dram_tensor("outT", [E, CHUNK], f32, kind="ExternalOutput")

    with TileContext(nc) as tc:
        with (
            tc.tile_pool(name="const", bufs=1) as cpool,
            tc.tile_pool(name="big", bufs=1) as bigpool,
            tc.tile_pool(name="work", bufs=3) as wpool,
        ):
            xb_t = bigpool.tile([E, S], f32, name="xb_t")
            xq_t = bigpool.tile([E, CHUNK], f32, name="xq_t")
            nc.sync.dma_start(xb_t[:], xb[:])
            nc.sync.dma_start(xq_t[:], xq[:])
            wq_t = cpool.tile([E, E], f32, name="wq_t")
            wk_t = cpool.tile([E, E], f32, name="wk_t")
            wv_t = cpool.tile([E, E], f32, name="wv_t")
            bq_t = cpool.tile([E, 1], f32, name="bq_t")
            bk_t = cpool.tile([E, 1], f32, name="bk_t")
            nc.sync.dma_start(wq_t[:], wq[:])
            nc.sync.dma_start(wk_t[:], wk[:])
            nc.sync.dma_start(wv_t[:], wv[:])
            nc.sync.dma_start(bq_t[:], bq[:])
            nc.sync.dma_start(bk_t[:], bk[:])
            ones_t = cpool.tile([128, 128], f16, name="ones_t")
            nc.vector.memset(ones_t[:], 1.0)

            qT = bigpool.tile([E, CHUNK], lp, name="qT")
            kT = bigpool.tile([E, S], lp, name="kT")
            vt = bigpool.tile([E, S], lp, name="vt")  # vt[:,128t:] = V tile t

            with tc.tile_pool(name="ps_proj", bufs=2, space="PSUM") as ppool:
                for j in range(NSB):
                    ps = ppool.tile([128, SBLK], f32, tag="proj")
                    nc.tensor.matmul(
                        ps[:],
                        wq_t[:],
                        xq_t[:, j * SBLK : (j + 1) * SBLK],
                        start=True,
                        stop=True,
                    )
                    nc.vector.tensor_scalar_add(
                        qT[:, j * SBLK : (j + 1) * SBLK], ps[:], bq_t[:, 0:1]
                    )
                for j in range(S // SBLK):
                    ps = ppool.tile([128, SBLK], f32, tag="proj")
                    nc.tensor.matmul(
                        ps[:],
                        wk_t[:],
                        xb_t[:, j * SBLK : (j + 1) * SBLK],
                        start=True,
                        stop=True,
                    )
                    nc.vector.tensor_scalar_add(
                        kT[:, j * SBLK : (j + 1) * SBLK], ps[:], bk_t[:, 0:1]
                    )
                for t in range(NT):
                    ps = ppool.tile([128, 128], f32, tag="projv")
                    nc.tensor.matmul(
                        ps[:],
                        xb_t[:, t * 128 : (t + 1) * 128],
                        wv_t[:],
                        start=True,
                        stop=True,
                    )
                    nc.vector.tensor_copy(vt[:, t * 128 : (t + 1) * 128], ps[:])

            with (
                tc.tile_pool(name="ps_s", bufs=3, space="PSUM") as spool,
                tc.tile_pool(name="ps_acc", bufs=1, space="PSUM") as apool,
            ):
                for half in range(2):
                    po = [
                        apool.tile([128, SBLK], f32, tag=f"po{i}", name=f"po{i}")
                        for i in range(2)
                    ]
                    pd = [
                        apool.tile([128, SBLK], f32, tag=f"pd{i}", name=f"pd{i}")
                        for i in range(2)
                    ]
                    for t in range(NT):
                        ktile = kT[:, t * 128 : (t + 1) * 128]
                        vtile = vt[:, t * 128 : (t + 1) * 128]
                        ptiles = []
                        for i in range(2):
                            sb = half * 2 + i
                            pss = spool.tile([128, SBLK], f32, tag="scores")
                            nc.tensor.matmul(
                                pss[:],
                                ktile,
                                qT[:, sb * SBLK : (sb + 1) * SBLK],
                                start=True,
                                stop=True,
                            )
                            pt = wpool.tile([128, SBLK], lp, tag="p")
                            nc.scalar.activation(pt[:], pss[:], Act.Exp)
                            ptiles.append(pt)
                        for i in range(2):
                            nc.tensor.matmul(
                                po[i][:],
                                vtile,
                                ptiles[i][:],
                                start=(t == 0),
                                stop=(t == NT - 1),
                            )
                        for i in range(2):
                            p16 = ptiles[i]
                            if PRECISION != "fast":
                                p16c = wpool.tile([128, SBLK], f16, tag="p16")
                                nc.vector.tensor_copy(p16c[:], ptiles[i][:])
                                p16 = p16c
                            nc.tensor.matmul(
                                pd[i][:],
                                ones_t[:],
                                p16[:],
                                start=(t == 0),
                                stop=(t == NT - 1),
                            )
                    for i in range(2):
                        sb = half * 2 + i
                        rc = wpool.tile([128, SBLK], f32, tag="rc")
                        nc.vector.reciprocal(rc[:], pd[i][:])
                        ot = wpool.tile([128, SBLK], f32, tag="ot")
                        nc.vector.tensor_mul(ot[:], po[i][:], rc[:])
                        nc.sync.dma_start(out[:, sb * SBLK : (sb + 1) * SBLK], ot[:])

    nc.compile()
    return nc


def _get_runner():
    """Build (once) and return a function in_maps -> list of per-core output
    dicts, with the jax.jit executable cached across calls."""
    if "runner" in _CACHE:
        return _CACHE["runner"]

    import jax
    import concourse.mybir as mybir
    from concourse import bass2jax
    from jax.experimental.shard_map import shard_map
    from jax.sharding import Mesh, PartitionSpec

    nc = _build_nc()
    bass2jax.install_neuronx_cc_hook()

    partition_name = nc.partition_id_tensor.name if nc.partition_id_tensor else None
    in_names = []
    out_names = []
    out_avals = []
    zero_shapes = []
    for alloc in nc.m.functions[0].allocations:
        if not isinstance(alloc, mybir.MemoryLocationSet):
            continue
        name = alloc.memorylocations[0].name
        if alloc.kind == "ExternalInput":
            if name != partition_name:
                in_names.append(name)
        elif alloc.kind == "ExternalOutput":
            shape = tuple(alloc.tensor_shape)
            dtype = mybir.dt.np(alloc.dtype)
            out_names.append(name)
            out_avals.append(jax.core.ShapedArray(shape, dtype))
            zero_shapes.append((shape, dtype))
    n_params = len(in_names)
    n_outs = len(out_names)
    all_in_names = list(in_names) + list(out_names)
    if partition_name is not None:
        all_in_names.append(partition_name)

    donate = tuple(range(n_params, n_params + n_outs))

    def _body(*args):
        operands = list(args)
        if partition_name is not None:
            operands.append(bass2jax.partition_id_tensor())
        outs = bass2jax._bass_exec_p.bind(
            *operands,
            out_avals=tuple(out_avals),
            in_names=tuple(all_in_names),
            out_names=tuple(out_names),
            lowering_input_output_aliases=(),
            sim_require_finite=True,
            sim_require_nnan=True,
            nc=nc,
        )
        return tuple(outs)

    devices = jax.devices()[:NCORES]
    mesh = Mesh(np.asarray(devices), ("core",))
    in_specs = (PartitionSpec("core"),) * (n_params + n_outs)
    out_specs = (PartitionSpec("core"),) * n_outs
    sharded = jax.jit(
        shard_map(
            _body, mesh=mesh, in_specs=in_specs, out_specs=out_specs, check_rep=False
        ),
        donate_argnums=donate,
        keep_unused=True,
    )

    def run(in_maps):
        concat_in = [
            np.concatenate([m[name] for m in in_maps], axis=0) for name in in_names
        ]
        concat_zeros = [
            np.zeros((NCORES * s[0], *s[1:]), d) for (s, d) in zero_shapes
        ]
        out_arrs = sharded(*concat_in, *concat_zeros)
        return [
            {
                name: np.asarray(out_arrs[i]).reshape(NCORES, *out_avals[i].shape)[c]
                for i, name in enumerate(out_names)
            }
            for c in range(NCORES)
        ]

    _CACHE["runner"] = run
    _CACHE["nc"] = nc
    return run


def _make_in_maps(x, Wq, bq, Wk, bk, Wv):
    wq_s = np.ascontiguousarray(Wq.T * SCALE).astype(np.float32)
    wk_t = np.ascontiguousarray(Wk.T).astype(np.float32)
    wv_t = np.ascontiguousarray(Wv.T).astype(np.float32)
    bq_s = (bq * SCALE).astype(np.float32).reshape(E, 1)
    bk_c = np.asarray(bk, dtype=np.float32).reshape(E, 1)
    in_maps = []
    for c in range(NCORES):
        b, sc = divmod(c, 2)
        xb = np.ascontiguousarray(x[b]).astype(np.float32)
        xq = np.ascontiguousarray(x[b][:, sc * CHUNK : (sc + 1) * CHUNK]).astype(
            np.float32
        )
        in_maps.append(
            {
                "xb": xb,
                "xq": xq,
                "wq": wq_s,
                "wk": wk_t,
                "wv": wv_t,
                "bq": bq_s,
                "bk": bk_c,
            }
        )
    return in_maps


def kernel(x, Wq, bq, Wk, bk, Wv, bv):
    x = np.asarray(x, dtype=np.float32)
    run = _get_runner()
    in_maps = _make_in_maps(x, Wq, bq, Wk, bk, Wv)
    results = run(in_maps)
    out = np.empty((B, S, E), dtype=np.float32)
    for c in range(NCORES):
        b, sc = divmod(c, 2)
        out[b, sc * CHUNK : (sc + 1) * CHUNK, :] = results[c]["outT"].T
    out += np.asarray(bv, dtype=np.float32)[None, None, :]
    return out


def run_traced(x, Wq, bq, Wk, bk, Wv, bv):
    """Like kernel() but via run_bass_kernel_spmd(trace=True); returns
    (out, exec_time_ns). Used by test.py for HW timing."""
    from concourse.bass_utils import run_bass_kernel_spmd

    if "nc" not in _CACHE:
        _get_runner()
    nc = _CACHE["nc"]
    in_maps = _make_in_maps(
        np.asarray(x, dtype=np.float32), Wq, bq, Wk, bk, Wv
    )
    res = run_bass_kernel_spmd(
        nc, in_maps, list(range(NCORES)), trace=True
    )
    out = np.empty((B, S, E), dtype=np.float32)
    for c in range(NCORES):
        b, sc = divmod(c, 2)
        out[b, sc * CHUNK : (sc + 1) * CHUNK, :] = res.results[c]["outT"].T
    out += np.asarray(bv, dtype=np.float32)[None, None, :]
    return out, res.exec_time_ns
